# revision 12
# baseline (speedup 1.0000x reference)
"""Self-contained Trainium2 Bass kernel: DeBERTa-style disentangled MHA.

Model (per reference):
    q = x @ Wq.T ; k = x @ Wk.T ; v = x @ Wv.T      (biases are zero)
    pos_k = rel_emb @ Wk.T ; pos_q = rel_emb @ Wq.T
    scores[i,j] = (q_i.k_j + A[i, i-j+s] + B[j, i-j+s]) * scale + mask[j]
        where A[i,t] = q_i . pos_k[t],  B[j,t] = k_j . pos_q[t]
    out = softmax_j(scores) @ v

Sharding: 8-way head-parallel (2 heads/core), every core handles all 8 batch rows.
Scores are computed transposed (k index on partitions) so probs feed the PV matmul
directly; the softmax denominator comes from an appended ones-column on V.
The relative-position diagonal gathers ("shear") go through a DRAM round trip in
bf16: per 128-row tile one 640-wide window is written contiguously (row pitch 640)
and read back with row pitch 639, turning the per-row relative shift into a plain
strided DMA that yields the full 512-wide j-range in one read per segment.
The per-batch pipeline interleaves projection, window, and score work so the
tensor engine stays continuously busy (p-state) while the round trips fly.
"""

import numpy as np

B, S, DIM, H, HD = 8, 512, 1024, 16, 64
NCORES = 8
HPC = H // NCORES            # heads per core = 2
SCALE = float((HD * 3) ** -0.5)
W = 640                      # shear window width per 128-row tile
SEG = W * 128                # flat DRAM elems per (m, I) segment

_prog_cache = {}


def _build_program():
    import concourse.bass as bass
    import concourse.mybir as mybir
    import concourse.tile as tile
    from concourse import bacc
    from concourse.masks import make_identity

    BF = mybir.dt.bfloat16
    F8 = mybir.dt.float8e4
    F32 = mybir.dt.float32
    AO = mybir.AluOpType
    AF = mybir.ActivationFunctionType

    nc = bacc.Bacc(None, target_bir_lowering=False, debug=False)

    def ap_of(t, extra_off, dims):
        return bass.AP(t.tensor, int(t.offset) + extra_off, dims)

    names = {}

    with tile.TileContext(nc) as tc:
        with tc.tile_pool(name="dram", bufs=1, space="DRAM") as dram, \
             tc.tile_pool(name="const", bufs=1) as const, \
             tc.tile_pool(name="persist", bufs=1) as persist:

            # ---------------- I/O ----------------
            xT_d = dram.tile([DIM, B * S], BF, kind="ExternalInput", name="xT")
            relT_d = dram.tile([DIM, 2 * S], BF, kind="ExternalInput", name="relT")
            wqT_d = dram.tile([DIM, 128], BF, kind="ExternalInput", name="wqT")
            wkT_d = dram.tile([DIM, 128], BF, kind="ExternalInput", name="wkT")
            wvT_d = dram.tile([DIM, 128], BF, kind="ExternalInput", name="wvT")
            mask_d = dram.tile([B, S], F32, kind="ExternalInput", name="mask")
            out_d = dram.tile([B * HPC, HD + 1, S], F32, kind="ExternalOutput",
                              name="out")
            for k, t in [("xT", xT_d), ("relT", relT_d), ("wqT", wqT_d),
                         ("wkT", wkT_d), ("wvT", wvT_d), ("mask", mask_d),
                         ("out", out_d)]:
                names[k] = t.name

            # ---------------- persistent SBUF ----------------
            ident = const.tile([128, 128], BF)
            make_identity(nc, ident)
            ident8 = const.tile([128, 128], F8)
            make_identity(nc, ident8)
            # mask_sb[p, b*4+J] = mask[b, 128J + p]
            mask_sb = const.tile([128, B, 4], F32)

            QT = persist.tile([128, B * S], BF)       # (x@WqT)*scale, transposed
            KT = persist.tile([128, B * S], BF)       # x@WkT, transposed
            posKTr = persist.tile([128, 2 * S], BF)   # pos_k^T, t-axis reversed
            posQT = persist.tile([128, 2 * S], BF)    # (pos_q^T)*scale
            # Vaug[:, b*4+J, 65h : 65h+65] = [v rows | ones] for PV lhsT
            Vaug = persist.tile([128, B * 4, 130], BF)
            nc.vector.memset(Vaug[:, :, 64:65], 1.0)
            nc.vector.memset(Vaug[:, :, 129:130], 1.0)

            with tc.tile_pool(name="wpool", bufs=1) as wpool, \
                 tc.tile_pool(name="sbw", bufs=1) as sbw, \
                 tc.tile_pool(name="dscratch", bufs=1, space="DRAM") as dscratch, \
                 tc.tile_pool(name="ps512", bufs=2, space="PSUM") as ps512, \
                 tc.tile_pool(name="psab", bufs=2, space="PSUM") as psab, \
                 tc.tile_pool(name="pspv", bufs=1, space="PSUM") as pspv:

                # ---------------- input loads ----------------
                # DMA issue order puts the pos-proj deps and the first x tile
                # ahead so the PE starts early and proj(0) isn't load-gated.
                wq_sb = wpool.tile([128, 8, 128], BF)
                wk_sb = wpool.tile([128, 8, 128], BF)
                wv_sb = wpool.tile([128, 8, 128], BF)
                nc.sync.dma_start(
                    out=wk_sb, in_=wkT_d.rearrange("(k p) o -> p k o", p=128))
                nc.sync.dma_start(
                    out=wq_sb, in_=wqT_d.rearrange("(k p) o -> p k o", p=128))
                xst = []

                def load_x(st):
                    t = sbw.tile([128, 8, 512], BF, name=f"xst{st}", tag="xst",
                                 bufs=4)
                    nc.sync.dma_start(
                        out=t,
                        in_=ap_of(xT_d, 512 * st,
                                  [[B * S, 128], [128 * B * S, 8], [1, 512]]))
                    xst.append(t)

                # ---------------- pos projections ----------------
                with tc.tile_pool(name="relpool", bufs=1) as relpool:
                    relch = []
                    for k in range(8):
                        t = relpool.tile([128, 2 * S], BF, name=f"relch{k}",
                                         tag=f"relch{k}")
                        nc.sync.dma_start(out=t,
                                          in_=relT_d[128 * k:128 * k + 128, :])
                        relch.append(t)
                        if k == 3:
                            load_x(0)
                        if k == 5:
                            nc.sync.dma_start(
                                out=wv_sb,
                                in_=wvT_d.rearrange("(k p) o -> p k o", p=128))
                    nc.sync.dma_start(
                        out=mask_sb,
                        in_=ap_of(mask_d, 0, [[1, 128], [S, B], [128, 4]]))
                    for st in range(1, 8):
                        load_x(st)

                    posKT_tmp = relpool.tile([128, 2 * S], BF)
                    for tt in range(2):
                        sl = slice(512 * tt, 512 * tt + 512)
                        pspk = psab.tile([128, 512], F32, tag="psAB")
                        for k in range(8):
                            fl = dict(start=(k == 0), stop=(k == 7))
                            nc.tensor.matmul(pspk, wk_sb[:, k, :],
                                             relch[k][:, sl], **fl)
                        nc.vector.tensor_copy(posKT_tmp[:, sl], pspk)
                        pspq = psab.tile([128, 512], F32, tag="psAB")
                        for k in range(8):
                            fl = dict(start=(k == 0), stop=(k == 7))
                            nc.tensor.matmul(pspq, wq_sb[:, k, :],
                                             relch[k][:, sl], **fl)
                        nc.vector.tensor_copy(posQT[:, sl], pspq)
                    # reversed copy: posKTr[:, t] = posKT_tmp[:, 1023 - t]
                    nc.vector.tensor_copy(
                        posKTr,
                        ap_of(posKT_tmp, 2 * S - 1, [[2 * S, 128], [-1, 2 * S]]))

                # ---------------- per-batch pipeline stages ----------------
                def emit_proj(b):
                    """q/k/v projection for tokens [512b, 512b+512) + Vaug."""
                    sl = slice(512 * b, 512 * b + 512)
                    psq = ps512.tile([128, 512], F32, tag="ps")
                    for k in range(8):
                        fl = dict(start=(k == 0), stop=(k == 7))
                        nc.tensor.matmul(psq, wq_sb[:, k, :], xst[b][:, k, :], **fl)
                    nc.vector.tensor_copy(QT[:, sl], psq)
                    psk = ps512.tile([128, 512], F32, tag="ps")
                    for k in range(8):
                        fl = dict(start=(k == 0), stop=(k == 7))
                        nc.tensor.matmul(psk, wk_sb[:, k, :], xst[b][:, k, :], **fl)
                    nc.scalar.copy(KT[:, sl], psk)
                    psv = ps512.tile([128, 512], F32, tag="ps")
                    for k in range(8):
                        fl = dict(start=(k == 0), stop=(k == 7))
                        nc.tensor.matmul(psv, wv_sb[:, k, :], xst[b][:, k, :], **fl)
                    vsb = sbw.tile([128, 512], BF, name="vsb", tag="vsb", bufs=2)
                    nc.vector.tensor_copy(vsb, psv)
                    # V transpose: j rows onto partitions
                    psvt = ps512.tile([128, 512], F32, tag="ps")
                    for J in range(4):
                        nc.tensor.matmul(psvt[:, 128 * J:128 * J + 128],
                                         vsb[:, 128 * J:128 * J + 128], ident,
                                         start=True, stop=True,
                                         skip_group_check=True)
                    for h in range(HPC):
                        nc.scalar.copy(
                            ap_of(Vaug, (4 * b) * 130 + 65 * h,
                                  [[32 * 130, 128], [130, 4], [1, 64]]),
                            ap_of(psvt, 64 * h,
                                  [[512, 128], [128, 4], [1, 64]]))

                def emit_ab(b):
                    """A/B windows -> psum -> SBUF -> DRAM -> sheared gathers.

                    window for row-tile I covers w0 = 384-128I:
                      A[p, w] = q_{128I+p} . posk[639 + 128I - w]   (reversed)
                      B[p, w] = k_{128I+p} . posq[(384-128I) + w]
                    """
                    ABsb, abflat, gath = {}, {}, {}
                    for h in range(HPC):
                        ABsb[h] = sbw.tile([128, 8, W], F8, name=f"ABsb{h}",
                                           tag=f"ABsb{h}", bufs=3)
                        abflat[h] = dscratch.tile([8 * SEG], F8,
                                                  name=f"abflat{h}",
                                                  tag=f"abflat{h}", bufs=3)
                        gath[h] = sbw.tile([128, 2, 4, 512], F8, name=f"gath{h}",
                                           tag=f"gath{h}", bufs=4)
                    for m in range(2):
                        lhs = QT if m == 0 else KT
                        rhs = posKTr if m == 0 else posQT
                        for I in range(4):
                            w0 = 384 - 128 * I
                            for h in range(HPC):
                                hp = slice(64 * h, 64 * h + 64)
                                ps = psab.tile([128, W], F32, name="psAB",
                                               tag="psAB", bufs=2)
                                lw = lhs[hp, 512 * b + 128 * I:
                                         512 * b + 128 * I + 128]
                                nc.tensor.matmul(ps[:, 0:512], lw,
                                                 rhs[hp, w0:w0 + 512],
                                                 start=True, stop=True,
                                                 tile_position=(64 * h, 0))
                                nc.tensor.matmul(ps[:, 512:W], lw,
                                                 rhs[hp, w0 + 512:w0 + W],
                                                 start=True, stop=True,
                                                 skip_group_check=True,
                                                 tile_position=(64 * h, 0))
                                seg = m * 4 + I
                                if (I + m) % 2 == 0:
                                    nc.vector.tensor_copy(ABsb[h][:, seg, :], ps)
                                else:
                                    nc.scalar.copy(ABsb[h][:, seg, :], ps)
                        # per-half write + gather read: halves the round-trip
                        # latency (A half feeds c2p, B half feeds p2c^T)
                        for h in range(HPC):
                            nc.gpsimd.dma_start(
                                out=ap_of(abflat[h], m * 4 * SEG,
                                          [[W, 128], [SEG, 4], [1, W]]),
                                in_=ABsb[h][:, 4 * m:4 * m + 4, :])
                            nc.sync.dma_start(
                                out=gath[h][:, m],
                                in_=ap_of(abflat[h],
                                          m * 4 * SEG + 127 + m,
                                          [[W - 1, 128], [SEG, 4], [1, 512]]))
                    return gath

                def emit_scores(b, gath, pvps):
                    for J in range(4):
                        qkps = {}
                        for h in range(HPC):
                            hp = slice(64 * h, 64 * h + 64)
                            qkps[h] = ps512.tile([128, 512], F32, name=f"qk{h}",
                                                 tag="ps")
                            nc.tensor.matmul(
                                qkps[h],
                                KT[hp, 512 * b + 128 * J: 512 * b + 128 * J + 128],
                                QT[hp, 512 * b: 512 * b + 512],
                                start=True, stop=False,
                                tile_position=(64 * h, 0))
                        for h in range(HPC):
                            for I in range(4):
                                nc.tensor.matmul(
                                    qkps[h][:, 128 * I:128 * I + 128],
                                    gath[h][:, 0, I, 128 * J:128 * J + 128],
                                    ident8, start=False, stop=(I == 3),
                                    skip_group_check=True)
                        for h in range(HPC):
                            PTp = sbw.tile([128, 512], BF, name=f"PTp{h}",
                                           tag=f"PTp{h}", bufs=2)
                            nc.vector.scalar_tensor_tensor(
                                PTp, qkps[h], mask_sb[:, b, J:J + 1],
                                gath[h][:, 1, J, :], AO.add, AO.add)
                            PT = sbw.tile([128, 512], BF, name=f"PT{h}",
                                          tag=f"PT{h}", bufs=2)
                            nc.scalar.activation(PT, PTp, AF.Exp)
                            nc.tensor.matmul(pvps[h],
                                             Vaug[:, 4 * b + J, 65 * h:65 * h + 65],
                                             PT, start=(J == 0), stop=(J == 3))

                def emit_out(b, pvps):
                    for h in range(HPC):
                        outsb = sbw.tile([65, 512], F32, name=f"outsb{h}",
                                         tag=f"outsb{h}", bufs=2)
                        nc.vector.tensor_copy(outsb, pvps[h])
                        nc.scalar.dma_start(out=out_d[HPC * b + h], in_=outsb)

                # ---------------- pipeline: lag scores by 2 batches ----------
                LAG = 2
                gaths = {}
                for b in range(B + LAG):
                    if b < B:
                        emit_proj(b)
                        gaths[b] = emit_ab(b)
                    if b >= LAG:
                        bb = b - LAG
                        pvps = {h: pspv.tile([65, 512], F32, name=f"pv{h}",
                                             tag=f"pv{h}", bufs=1)
                                for h in range(HPC)}
                        emit_scores(bb, gaths.pop(bb), pvps)
                        emit_out(bb, pvps)

    nc.compile()
    return nc, names


def _get_program():
    if "prog" not in _prog_cache:
        _prog_cache["prog"] = _build_program()
    return _prog_cache["prog"]


def _host_prep(x, rel_embeddings, attn_mask, Wq, bq, Wk, bk, Wv, bv):
    import ml_dtypes
    bf = ml_dtypes.bfloat16
    x = np.asarray(x, np.float32)
    xT = np.ascontiguousarray(x.reshape(B * S, DIM).T).astype(bf)
    relT = np.ascontiguousarray(np.asarray(rel_embeddings, np.float32).T).astype(bf)
    WqT = np.asarray(Wq, np.float32).T * SCALE   # fold softmax scale into q
    WkT = np.asarray(Wk, np.float32).T
    WvT = np.asarray(Wv, np.float32).T
    mask = np.ascontiguousarray(
        np.asarray(attn_mask, np.float32).reshape(B, S))
    maps = []
    for c in range(NCORES):
        sl = slice(128 * c, 128 * c + 128)
        maps.append({
            "xT": xT,
            "relT": relT,
            "wqT": np.ascontiguousarray(WqT[:, sl]).astype(bf),
            "wkT": np.ascontiguousarray(WkT[:, sl]).astype(bf),
            "wvT": np.ascontiguousarray(WvT[:, sl]).astype(bf),
            "mask": mask,
        })
    return maps


def kernel(x, rel_embeddings, attn_mask, Wq, bq, Wk, bk, Wv, bv):
    from concourse.bass_utils import run_bass_kernel_spmd

    nc, names = _get_program()
    maps = _host_prep(x, rel_embeddings, attn_mask, Wq, bq, Wk, bk, Wv, bv)
    in_maps = [{names[k]: v for k, v in m.items()} for m in maps]
    res = run_bass_kernel_spmd(nc, in_maps, list(range(NCORES)))
    out = np.empty((B, S, DIM), np.float32)
    for c in range(NCORES):
        o = np.asarray(res.results[c][names["out"]], np.float32)
        for b in range(B):
            for hl in range(HPC):
                d0 = 128 * c + 64 * hl
                blk = o[HPC * b + hl]          # [65, 512]: rows 0-63 PV, row 64 L
                out[b, :, d0:d0 + 64] = (blk[0:64] / blk[64:65]).T
    return out


# revision 16
# speedup vs baseline: 1.0030x; 1.0030x over previous
"""Self-contained Trainium2 Bass kernel: DeBERTa-style disentangled MHA.

Model (per reference):
    q = x @ Wq.T ; k = x @ Wk.T ; v = x @ Wv.T      (biases are zero)
    pos_k = rel_emb @ Wk.T ; pos_q = rel_emb @ Wq.T
    scores[i,j] = (q_i.k_j + A[i, i-j+s] + B[j, i-j+s]) * scale + mask[j]
        where A[i,t] = q_i . pos_k[t],  B[j,t] = k_j . pos_q[t]
    out = softmax_j(scores) @ v

Sharding: 8-way head-parallel (2 heads/core), every core handles all 8 batch rows.
Scores are computed transposed (k index on partitions) so probs feed the PV matmul
directly; the softmax denominator comes from an appended ones-column on V.
The relative-position diagonal gathers ("shear") go through a DRAM round trip in
bf16: per 128-row tile one 640-wide window is written contiguously (row pitch 640)
and read back with row pitch 639, turning the per-row relative shift into a plain
strided DMA that yields the full 512-wide j-range in one read per segment.
The per-batch pipeline interleaves projection, window, and score work so the
tensor engine stays continuously busy (p-state) while the round trips fly.
"""

import numpy as np

B, S, DIM, H, HD = 8, 512, 1024, 16, 64
NCORES = 8
HPC = H // NCORES            # heads per core = 2
SCALE = float((HD * 3) ** -0.5)
W = 640                      # shear window width per 128-row tile
SEG = W * 128                # flat DRAM elems per (m, I) segment

_prog_cache = {}


def _build_program():
    import concourse.bass as bass
    import concourse.mybir as mybir
    import concourse.tile as tile
    from concourse import bacc
    from concourse.masks import make_identity

    BF = mybir.dt.bfloat16
    F8 = mybir.dt.float8e4
    F32 = mybir.dt.float32
    AO = mybir.AluOpType
    AF = mybir.ActivationFunctionType

    nc = bacc.Bacc(None, target_bir_lowering=False, debug=False)

    def ap_of(t, extra_off, dims):
        return bass.AP(t.tensor, int(t.offset) + extra_off, dims)

    names = {}

    with tile.TileContext(nc) as tc:
        with tc.tile_pool(name="dram", bufs=1, space="DRAM") as dram, \
             tc.tile_pool(name="const", bufs=1) as const, \
             tc.tile_pool(name="persist", bufs=1) as persist:

            # ---------------- I/O ----------------
            xT_d = dram.tile([DIM, B * S], BF, kind="ExternalInput", name="xT")
            relT_d = dram.tile([DIM, 2 * S], BF, kind="ExternalInput", name="relT")
            wqT_d = dram.tile([DIM, 128], BF, kind="ExternalInput", name="wqT")
            wkT_d = dram.tile([DIM, 128], BF, kind="ExternalInput", name="wkT")
            wvT_d = dram.tile([DIM, 128], BF, kind="ExternalInput", name="wvT")
            mask_d = dram.tile([B, S], F32, kind="ExternalInput", name="mask")
            out_d = dram.tile([B * HPC, HD + 1, S], F32, kind="ExternalOutput",
                              name="out")
            for k, t in [("xT", xT_d), ("relT", relT_d), ("wqT", wqT_d),
                         ("wkT", wkT_d), ("wvT", wvT_d), ("mask", mask_d),
                         ("out", out_d)]:
                names[k] = t.name

            # ---------------- persistent SBUF ----------------
            ident = const.tile([128, 128], BF)
            make_identity(nc, ident)
            ident8 = const.tile([128, 128], F8)
            make_identity(nc, ident8)
            # mask_sb[p, b*4+J] = mask[b, 128J + p]
            mask_sb = const.tile([128, B, 4], F32)

            QT = persist.tile([128, B * S], BF)       # (x@WqT)*scale, transposed
            KT = persist.tile([128, B * S], BF)       # x@WkT, transposed
            posKTr = persist.tile([128, 2 * S], BF)   # pos_k^T, t-axis reversed
            posQT = persist.tile([128, 2 * S], BF)    # (pos_q^T)*scale
            # Vaug[:, b*4+J, 65h : 65h+65] = [v rows | ones] for PV lhsT
            Vaug = persist.tile([128, B * 4, 130], BF)
            nc.vector.memset(Vaug[:, :, 64:65], 1.0)
            nc.vector.memset(Vaug[:, :, 129:130], 1.0)

            with tc.tile_pool(name="wpool", bufs=1) as wpool, \
                 tc.tile_pool(name="sbw", bufs=1) as sbw, \
                 tc.tile_pool(name="dscratch", bufs=1, space="DRAM") as dscratch, \
                 tc.tile_pool(name="ps512", bufs=2, space="PSUM") as ps512, \
                 tc.tile_pool(name="psab", bufs=2, space="PSUM") as psab, \
                 tc.tile_pool(name="pspv", bufs=1, space="PSUM") as pspv:

                # ---------------- input loads ----------------
                # DMA issue order puts the pos-proj deps and the first x tile
                # ahead so the PE starts early and proj(0) isn't load-gated.
                wq_sb = wpool.tile([128, 8, 128], BF)
                wk_sb = wpool.tile([128, 8, 128], BF)
                wv_sb = wpool.tile([128, 8, 128], BF)
                nc.sync.dma_start(
                    out=wk_sb, in_=wkT_d.rearrange("(k p) o -> p k o", p=128))
                nc.sync.dma_start(
                    out=wq_sb, in_=wqT_d.rearrange("(k p) o -> p k o", p=128))
                xst = []

                def load_x(st):
                    t = sbw.tile([128, 8, 512], BF, name=f"xst{st}", tag="xst",
                                 bufs=8)
                    nc.sync.dma_start(
                        out=t,
                        in_=ap_of(xT_d, 512 * st,
                                  [[B * S, 128], [128 * B * S, 8], [1, 512]]))
                    xst.append(t)

                # ---------------- pos projections ----------------
                with tc.tile_pool(name="relpool", bufs=1) as relpool:
                    relch = []
                    for k in range(8):
                        t = relpool.tile([128, 2 * S], BF, name=f"relch{k}",
                                         tag=f"relch{k}")
                        nc.sync.dma_start(out=t,
                                          in_=relT_d[128 * k:128 * k + 128, :])
                        relch.append(t)
                        if k == 3:
                            load_x(0)
                        if k == 5:
                            nc.sync.dma_start(
                                out=wv_sb,
                                in_=wvT_d.rearrange("(k p) o -> p k o", p=128))
                    nc.sync.dma_start(
                        out=mask_sb,
                        in_=ap_of(mask_d, 0, [[1, 128], [S, B], [128, 4]]))
                    for st in range(1, 8):
                        load_x(st)

                    posKT_tmp = relpool.tile([128, 2 * S], BF)
                    for tt in range(2):
                        sl = slice(512 * tt, 512 * tt + 512)
                        pspk = ps512.tile([128, 512], F32, tag="ps")
                        for k in range(8):
                            fl = dict(start=(k == 0), stop=(k == 7))
                            nc.tensor.matmul(pspk, wk_sb[:, k, :],
                                             relch[k][:, sl], **fl)
                        nc.vector.tensor_copy(posKT_tmp[:, sl], pspk)
                        pspq = ps512.tile([128, 512], F32, tag="ps")
                        for k in range(8):
                            fl = dict(start=(k == 0), stop=(k == 7))
                            nc.tensor.matmul(pspq, wq_sb[:, k, :],
                                             relch[k][:, sl], **fl)
                        nc.vector.tensor_copy(posQT[:, sl], pspq)
                    # reversed copy: posKTr[:, t] = posKT_tmp[:, 1023 - t]
                    nc.vector.tensor_copy(
                        posKTr,
                        ap_of(posKT_tmp, 2 * S - 1, [[2 * S, 128], [-1, 2 * S]]))

                # ---------------- per-batch pipeline stages ----------------
                def emit_proj(b):
                    """q/k/v projection for tokens [512b, 512b+512) + Vaug."""
                    sl = slice(512 * b, 512 * b + 512)
                    psq = ps512.tile([128, 512], F32, tag="ps")
                    for k in range(8):
                        fl = dict(start=(k == 0), stop=(k == 7))
                        nc.tensor.matmul(psq, wq_sb[:, k, :], xst[b][:, k, :], **fl)
                    nc.vector.tensor_copy(QT[:, sl], psq)
                    psk = ps512.tile([128, 512], F32, tag="ps")
                    for k in range(8):
                        fl = dict(start=(k == 0), stop=(k == 7))
                        nc.tensor.matmul(psk, wk_sb[:, k, :], xst[b][:, k, :], **fl)
                    nc.scalar.copy(KT[:, sl], psk)
                    psv = ps512.tile([128, 512], F32, tag="ps")
                    for k in range(8):
                        fl = dict(start=(k == 0), stop=(k == 7))
                        nc.tensor.matmul(psv, wv_sb[:, k, :], xst[b][:, k, :], **fl)
                    vsb = sbw.tile([128, 512], BF, name="vsb", tag="vsb", bufs=2)
                    nc.vector.tensor_copy(vsb, psv)
                    # V transpose: j rows onto partitions
                    psvt = ps512.tile([128, 512], F32, tag="ps")
                    for J in range(4):
                        nc.tensor.matmul(psvt[:, 128 * J:128 * J + 128],
                                         vsb[:, 128 * J:128 * J + 128], ident,
                                         start=True, stop=True,
                                         skip_group_check=True)
                    for h in range(HPC):
                        nc.scalar.copy(
                            ap_of(Vaug, (4 * b) * 130 + 65 * h,
                                  [[32 * 130, 128], [130, 4], [1, 64]]),
                            ap_of(psvt, 64 * h,
                                  [[512, 128], [128, 4], [1, 64]]))

                def emit_ab(b):
                    """A/B windows -> psum -> SBUF -> DRAM -> sheared gathers.

                    window for row-tile I covers w0 = 384-128I:
                      A[p, w] = q_{128I+p} . posk[639 + 128I - w]   (reversed)
                      B[p, w] = k_{128I+p} . posq[(384-128I) + w]
                    """
                    ABsb, abflat, gath = {}, {}, {}
                    for h in range(HPC):
                        ABsb[h] = sbw.tile([128, 8, W], F8, name=f"ABsb{h}",
                                           tag=f"ABsb{h}", bufs=3)
                        abflat[h] = dscratch.tile([8 * SEG], F8,
                                                  name=f"abflat{h}",
                                                  tag=f"abflat{h}", bufs=3)
                        gath[h] = sbw.tile([128, 2, 4, 512], F8, name=f"gath{h}",
                                           tag=f"gath{h}", bufs=4)
                    for m in range(2):
                        lhs = QT if m == 0 else KT
                        rhs = posKTr if m == 0 else posQT
                        for I in range(4):
                            w0 = 384 - 128 * I
                            for h in range(HPC):
                                hp = slice(64 * h, 64 * h + 64)
                                ps = psab.tile([128, W], F32, name="psAB",
                                               tag="psAB", bufs=2)
                                lw = lhs[hp, 512 * b + 128 * I:
                                         512 * b + 128 * I + 128]
                                nc.tensor.matmul(ps[:, 0:512], lw,
                                                 rhs[hp, w0:w0 + 512],
                                                 start=True, stop=True,
                                                 tile_position=(64 * h, 0))
                                nc.tensor.matmul(ps[:, 512:W], lw,
                                                 rhs[hp, w0 + 512:w0 + W],
                                                 start=True, stop=True,
                                                 skip_group_check=True,
                                                 tile_position=(64 * h, 0))
                                seg = m * 4 + I
                                if (I + m) % 2 == 0:
                                    nc.vector.tensor_copy(ABsb[h][:, seg, :], ps)
                                else:
                                    nc.scalar.copy(ABsb[h][:, seg, :], ps)
                        # per-half write + gather read: halves the round-trip
                        # latency (A half feeds c2p, B half feeds p2c^T)
                        for h in range(HPC):
                            nc.gpsimd.dma_start(
                                out=ap_of(abflat[h], m * 4 * SEG,
                                          [[W, 128], [SEG, 4], [1, W]]),
                                in_=ABsb[h][:, 4 * m:4 * m + 4, :])
                            nc.scalar.dma_start(
                                out=gath[h][:, m],
                                in_=ap_of(abflat[h],
                                          m * 4 * SEG + 127 + m,
                                          [[W - 1, 128], [SEG, 4], [1, 512]]))
                    return gath

                def emit_scores(b, gath, pvps):
                    for J in range(4):
                        qkps = {}
                        for h in range(HPC):
                            hp = slice(64 * h, 64 * h + 64)
                            qkps[h] = ps512.tile([128, 512], F32, name=f"qk{h}",
                                                 tag="ps")
                            nc.tensor.matmul(
                                qkps[h],
                                KT[hp, 512 * b + 128 * J: 512 * b + 128 * J + 128],
                                QT[hp, 512 * b: 512 * b + 512],
                                start=True, stop=False,
                                tile_position=(64 * h, 0))
                        for h in range(HPC):
                            for I in range(4):
                                nc.tensor.matmul(
                                    qkps[h][:, 128 * I:128 * I + 128],
                                    gath[h][:, 0, I, 128 * J:128 * J + 128],
                                    ident8, start=False, stop=(I == 3),
                                    skip_group_check=True)
                        for h in range(HPC):
                            PTp = sbw.tile([128, 512], BF, name=f"PTp{h}",
                                           tag=f"PTp{h}", bufs=2)
                            nc.vector.scalar_tensor_tensor(
                                PTp, qkps[h], mask_sb[:, b, J:J + 1],
                                gath[h][:, 1, J, :], AO.add, AO.add)
                            PT = sbw.tile([128, 512], BF, name=f"PT{h}",
                                          tag=f"PT{h}", bufs=2)
                            nc.scalar.activation(PT, PTp, AF.Exp)
                            nc.tensor.matmul(pvps[h],
                                             Vaug[:, 4 * b + J, 65 * h:65 * h + 65],
                                             PT, start=(J == 0), stop=(J == 3))

                def emit_out(b, pvps):
                    for h in range(HPC):
                        outsb = sbw.tile([65, 512], F32, name=f"outsb{h}",
                                         tag=f"outsb{h}", bufs=2)
                        nc.vector.tensor_copy(outsb, pvps[h])
                        nc.sync.dma_start(out=out_d[HPC * b + h], in_=outsb)

                # ---------------- pipeline: lag scores by 2 batches ----------
                LAG = 2
                gaths = {}
                for b in range(B + LAG):
                    if b < B:
                        emit_proj(b)
                        gaths[b] = emit_ab(b)
                    if b >= LAG:
                        bb = b - LAG
                        pvps = {h: pspv.tile([65, 512], F32, name=f"pv{h}",
                                             tag=f"pv{h}", bufs=1)
                                for h in range(HPC)}
                        emit_scores(bb, gaths.pop(bb), pvps)
                        emit_out(bb, pvps)

    nc.compile()
    return nc, names


def _get_program():
    if "prog" not in _prog_cache:
        _prog_cache["prog"] = _build_program()
    return _prog_cache["prog"]


def _host_prep(x, rel_embeddings, attn_mask, Wq, bq, Wk, bk, Wv, bv):
    import ml_dtypes
    bf = ml_dtypes.bfloat16
    x = np.asarray(x, np.float32)
    xT = np.ascontiguousarray(x.reshape(B * S, DIM).T).astype(bf)
    relT = np.ascontiguousarray(np.asarray(rel_embeddings, np.float32).T).astype(bf)
    WqT = np.asarray(Wq, np.float32).T * SCALE   # fold softmax scale into q
    WkT = np.asarray(Wk, np.float32).T
    WvT = np.asarray(Wv, np.float32).T
    mask = np.ascontiguousarray(
        np.asarray(attn_mask, np.float32).reshape(B, S))
    maps = []
    for c in range(NCORES):
        sl = slice(128 * c, 128 * c + 128)
        maps.append({
            "xT": xT,
            "relT": relT,
            "wqT": np.ascontiguousarray(WqT[:, sl]).astype(bf),
            "wkT": np.ascontiguousarray(WkT[:, sl]).astype(bf),
            "wvT": np.ascontiguousarray(WvT[:, sl]).astype(bf),
            "mask": mask,
        })
    return maps


def kernel(x, rel_embeddings, attn_mask, Wq, bq, Wk, bk, Wv, bv):
    from concourse.bass_utils import run_bass_kernel_spmd

    nc, names = _get_program()
    maps = _host_prep(x, rel_embeddings, attn_mask, Wq, bq, Wk, bk, Wv, bv)
    in_maps = [{names[k]: v for k, v in m.items()} for m in maps]
    res = run_bass_kernel_spmd(nc, in_maps, list(range(NCORES)))
    out = np.empty((B, S, DIM), np.float32)
    for c in range(NCORES):
        o = np.asarray(res.results[c][names["out"]], np.float32)
        for b in range(B):
            for hl in range(HPC):
                d0 = 128 * c + 64 * hl
                blk = o[HPC * b + hl]          # [65, 512]: rows 0-63 PV, row 64 L
                out[b, :, d0:d0 + 64] = (blk[0:64] / blk[64:65]).T
    return out
